# revision 34
# baseline (speedup 1.0000x reference)
"""EnhancedMultiHeadAttention on 8 TRN2 NeuronCores — v2.

Sharding: core c = (batch b=c//2, half hh=c%2) owns query rows
hh*1024:(hh+1)*1024 and the SAME kv token half. k/v projections are computed
for the own token half only and exchanged with the pair core via pairwise
AllGather collectives (removes the baseline's duplicated k/v projection work).

Kernel structure per core (bf16 matmuls, f32 softmax/LN):
  - LN gain/beta and projection biases folded on the HOST (W' = diag(g)W,
    b' = beta@W + b); the device only applies bias paths when nonzero.
  - Scores run as TWO CONCURRENT K=64 row-tiled matmuls (head pair on
    partition halves 0:64 / 64:128 of the same kT/qT block) — 2x the
    padded-K=128 baseline throughput.
  - exp on the scalar engine (the attention-phase pacer, ~261us); program
    order is arranged so the exp stream starts as early as possible and
    projections/DMA/exchange overlap it.
  - A@V uses V with a ones column appended (M=65) so the softmax denominator
    accumulates for free in PSUM row 64; normalization via
    reciprocal_approx_fast + gpsimd partition broadcast.
  - Gate is folded into the final LN scale (LN(c*x) trick from baseline).
"""

import os
import numpy as np

D = 1024
H = 16
HD = 64
S = 2048
B = 4
SQ = 1024       # query rows per core
SK = 2048       # kv rows per batch
KT = D // 128   # contraction tiles
N_CORES = 8
EPS = 1e-5
PAIRS = [[0, 1], [2, 3], [4, 5], [6, 7]]

_CACHE = {}


def _build(dedup=True, triv_k=True, triv_q=True, triv_v=True, triv_o=True,
           triv_lno=True):
    from contextlib import ExitStack

    import concourse.bacc as bacc
    import concourse.bass as bass
    import concourse.mybir as mybir
    import concourse.tile as tile
    from concourse.masks import make_identity

    f32 = mybir.dt.float32
    bf16 = mybir.dt.bfloat16
    AF = mybir.ActivationFunctionType
    OP = mybir.AluOpType

    TH = SK // 2 if dedup else SK  # kv tokens projected locally
    TT = SK // 128                 # global kv token tiles (16)
    VROW = H * (HD + 1)            # 1040

    nc = bacc.Bacc("TRN2", target_bir_lowering=False, debug=False,
                   num_devices=N_CORES)

    xq = nc.dram_tensor("xq", [SQ, D], f32, kind="ExternalInput").ap()
    xk = nc.dram_tensor("xk", [TH, D], f32, kind="ExternalInput").ap()
    xv = nc.dram_tensor("xv", [TH, D], f32, kind="ExternalInput").ap()
    # weights arrive pre-cast to bf16 (device would cast anyway): halves the
    # weight DMA traffic and removes the cast ops + staging SBUF entirely.
    Wq_d = nc.dram_tensor("Wq", [D, D], bf16, kind="ExternalInput").ap()
    Wk_d = nc.dram_tensor("Wk", [D, D], bf16, kind="ExternalInput").ap()
    Wv_d = nc.dram_tensor("Wv", [D, D], bf16, kind="ExternalInput").ap()
    Wo_d = nc.dram_tensor("Wo", [D, D], bf16, kind="ExternalInput").ap()
    Wg_d = nc.dram_tensor("Wg", [KT, 128], bf16, kind="ExternalInput").ap()
    bqc_d = nc.dram_tensor("bqc", [128, KT], f32, kind="ExternalInput").ap()
    bkc_d = nc.dram_tensor("bkc", [128, KT], f32, kind="ExternalInput").ap()
    bvr_d = nc.dram_tensor("bvr", [1, D], f32, kind="ExternalInput").ap()
    bor_d = nc.dram_tensor("bor", [1, D], f32, kind="ExternalInput").ap()
    bg_d = nc.dram_tensor("bg", [1, 1], f32, kind="ExternalInput").ap()
    lnog_d = nc.dram_tensor("lnog", [1, D], f32, kind="ExternalInput").ap()
    lnob_d = nc.dram_tensor("lnob", [1, D], f32, kind="ExternalInput").ap()
    out_d = nc.dram_tensor("out", [SQ, D], f32, kind="ExternalOutput").ap()
    kdbg = bool(os.environ.get("KDBG"))
    if kdbg:
        dbg_k = nc.dram_tensor("dbg_k", [KT, 128, SK], f32,
                               kind="ExternalOutput").ap()
        dbg_q = nc.dram_tensor("dbg_q", [KT, 128, SQ], f32,
                               kind="ExternalOutput").ap()
        dbg_v = nc.dram_tensor("dbg_v", [SK // 128, 128, H * (HD + 1)], f32,
                               kind="ExternalOutput").ap()
        dbg_a = nc.dram_tensor("dbg_a", [KT, 128, SQ], f32,
                               kind="ExternalOutput").ap()
        dbg_g = nc.dram_tensor("dbg_g", [128, SQ // 128], f32,
                               kind="ExternalOutput").ap()

    def bcast_rows(ap2d, p):
        return bass.AP(tensor=ap2d.tensor, offset=ap2d.offset,
                       ap=[[0, p]] + list(ap2d.ap[1:]))

    with tile.TileContext(nc) as tc:
        with ExitStack() as ctx:
            const = ctx.enter_context(tc.tile_pool(name="const", bufs=1))
            main = ctx.enter_context(tc.tile_pool(name="main", bufs=1))

            identity = const.tile([128, 128], bf16)
            make_identity(nc, identity)
            eps_t = const.tile([128, 1], f32)
            nc.vector.memset(eps_t, EPS)
            ones_row = const.tile([1, 512], bf16)
            nc.vector.memset(ones_row, 1.0)
            if not triv_k:
                bkc_s = const.tile([128, KT], f32)
                nc.sync.dma_start(out=bkc_s, in_=bkc_d)
            if not triv_q:
                bqc_s = const.tile([128, KT], f32)
                nc.sync.dma_start(out=bqc_s, in_=bqc_d)
                bgf = const.tile([1, 1], f32)
                nc.sync.dma_start(out=bgf, in_=bg_d)
                bg_s = const.tile([1, 1], bf16)
                nc.vector.tensor_copy(out=bg_s, in_=bgf)

            # persistent per-core tiles
            kT_s = main.tile([128, KT, SK], bf16)
            qT_s = main.tile([128, KT, SQ], bf16)
            v_aug = main.tile([128, TT, H, HD + 1], bf16)
            attn_oT = main.tile([128, KT, SQ], bf16)
            gate_s = main.tile([128, SQ // 128], f32)

            if dedup:
                dram = ctx.enter_context(
                    tc.tile_pool(name="dram", bufs=1, space="DRAM"))
                k_bounce = [dram.tile([128, KT // 2 * TH], bf16,
                                      name=f"kb{i}") for i in range(2)]
                k_gath = [dram.tile([2, 128, KT // 2 * TH], bf16,
                                    name=f"kg{i}") for i in range(2)]
                v_bounce_a = dram.tile([128, 4 * VROW], bf16)
                v_gath_a = dram.tile([2, 128, 4 * VROW], bf16)
                v_bounce_b = dram.tile([128, 4 * VROW], bf16)
                v_gath_b = dram.tile([2, 128, 4 * VROW], bf16)

            def load_weight(w_dram, Ws):
                # direct bf16 DMA: Ws[p, k, d] = W[k*128+p, d]
                nc.sync.dma_start(
                    out=Ws, in_=w_dram.rearrange("(k p) d -> p k d", p=128))

            def ln_transpose(x_dram, n_tok, dst, lnw, lps, name):
                """LN (no gain/beta) + PE transpose into dst [128, KT, n_tok]."""
                for t in range(n_tok // 128):
                    xt = lnw.tile([128, D], f32, tag="x", name=f"{name}x{t}")
                    nc.sync.dma_start(
                        out=xt, in_=x_dram[t * 128:(t + 1) * 128, :])
                    xt3 = xt.rearrange("p (s f) -> p s f", s=2)
                    stats = lnw.tile([128, 2, 6], f32, tag="st")
                    nc.vector.bn_stats(out=stats[:, 0, :], in_=xt3[:, 0, :])
                    nc.vector.bn_stats(out=stats[:, 1, :], in_=xt3[:, 1, :])
                    mv = lnw.tile([128, 2], f32, tag="mv")
                    nc.vector.bn_aggr(out=mv, in_=stats)
                    rstd = lnw.tile([128, 1], f32, tag="rs")
                    nc.scalar.activation(out=rstd, in_=mv[:, 1:2],
                                         func=AF.Sqrt, bias=eps_t)
                    nc.vector.reciprocal(out=rstd, in_=rstd)
                    xc = lnw.tile([128, D], bf16, tag="xc", bufs=2)
                    nc.vector.tensor_scalar(
                        out=xc, in0=xt, scalar1=mv[:, 0:1], scalar2=rstd,
                        op0=OP.subtract, op1=OP.mult)
                    pt = lps.tile([128, KT, 128], bf16, tag="pt")
                    for c in range(KT):
                        nc.tensor.transpose(
                            out=pt[:, c, :],
                            in_=xc[:, c * 128:(c + 1) * 128],
                            identity=identity)
                    nc.scalar.copy(out=dst[:, :, t * 128:(t + 1) * 128],
                                   in_=pt)

            # ---------------- K path ----------------
            with tc.tile_pool(name="kstg", bufs=1) as kstg, \
                    tc.tile_pool(name="klnw", bufs=3) as lnw, \
                    tc.tile_pool(name="klps", bufs=2, space="PSUM") as lps, \
                    tc.tile_pool(name="kpps", bufs=3, space="PSUM") as pps:
                Wk_s = kstg.tile([128, KT, D], bf16, tag="W")
                load_weight(Wk_d, Wk_s)
                knT = kstg.tile([128, KT, TH], bf16, tag="xn")
                ln_transpose(xk, TH, knT, lnw, lps, "kn")
                dstk = kstg.tile([128, KT, TH], bf16, tag="kh",
                                 name="kT_half") if dedup else kT_s
                # exchange in two halves (m 0-3, m 4-7) so pair-0 scores can
                # start as soon as the first half lands.
                for half in range(2):
                    for m in range(half * KT // 2, (half + 1) * KT // 2):
                        for n in range(TH // 512):
                            ps = pps.tile([128, 512], f32, tag="pj",
                                          name=f"kp{m}_{n}")
                            for kt in range(KT):
                                nc.tensor.matmul(
                                    out=ps,
                                    lhsT=Wk_s[:, kt, m * 128:(m + 1) * 128],
                                    rhs=knT[:, kt, n * 512:(n + 1) * 512],
                                    start=(kt == 0), stop=(kt == KT - 1))
                            if triv_k:
                                nc.vector.tensor_copy(
                                    out=dstk[:, m, n * 512:(n + 1) * 512],
                                    in_=ps)
                            else:
                                nc.scalar.activation(
                                    out=dstk[:, m, n * 512:(n + 1) * 512],
                                    in_=ps, func=AF.Identity,
                                    bias=bkc_s[:, m:m + 1])
                        if dedup:
                            mm = m - half * KT // 2
                            nc.sync.dma_start(
                                out=k_bounce[half][:, mm * TH:(mm + 1) * TH],
                                in_=dstk[:, m, :])
                    if dedup:
                        nc.gpsimd.collective_compute(
                            "AllGather", OP.bypass, replica_groups=PAIRS,
                            ins=[k_bounce[half].opt()],
                            outs=[k_gath[half].opt()])
                        kg = k_gath[half].rearrange("s p (m t) -> p m s t",
                                                    m=KT // 2)
                        for mm in range(KT // 2):
                            nc.sync.dma_start(
                                out=kT_s[:, half * KT // 2 + mm, :].rearrange(
                                    "p (s t) -> p s t", s=2),
                                in_=kg[:, mm])

            # ---------------- Q prep (proj happens inside pair loop) -------
            qper = ctx.enter_context(tc.tile_pool(name="qper", bufs=1))
            qnT_t = qper.tile([128, KT, SQ], bf16)
            Wq_s = qper.tile([128, KT, D], bf16)
            Wg_s = qper.tile([128, KT], bf16)
            # zero-padded per-parity q staging: scores run K=128 against the
            # full two-head kT block (other head's rows hit zeros). K=64
            # matmuls get HAM-throttled to half clock — padding is faster.
            qtz = [qper.tile([128, SQ], bf16, name=f"qtz{i}")
                   for i in range(2)]
            nc.vector.memset(qtz[0], 0.0)
            nc.vector.memset(qtz[1], 0.0)
            with tc.tile_pool(name="qlnw", bufs=3) as lnw, \
                    tc.tile_pool(name="qlps", bufs=2, space="PSUM") as lps, \
                    tc.tile_pool(name="gps", bufs=1, space="PSUM") as gps:
                load_weight(Wq_d, Wq_s)
                nc.sync.dma_start(out=Wg_s,
                                  in_=Wg_d.rearrange("k p -> p k"))
                ln_transpose(xq, SQ, qnT_t, lnw, lps, "qn")
                for tt in range(SQ // 128):
                    ps = gps.tile([128, 1], f32, tag="g", name=f"g{tt}")
                    for kt in range(KT):
                        nc.tensor.matmul(
                            out=ps,
                            lhsT=qnT_t[:, kt, tt * 128:(tt + 1) * 128],
                            rhs=Wg_s[:, kt:kt + 1],
                            start=(kt == 0), stop=(kt == KT - 1 and triv_q))
                    if not triv_q:
                        nc.tensor.matmul(out=ps, lhsT=ones_row[:, 0:128],
                                         rhs=bg_s, start=False, stop=True)
                    nc.scalar.activation(out=gate_s[:, tt:tt + 1], in_=ps,
                                         func=AF.Sigmoid)

            # ---------------- V path ----------------
            with tc.tile_pool(name="vstg", bufs=1) as vstg, \
                    tc.tile_pool(name="vlnw", bufs=3) as lnw, \
                    tc.tile_pool(name="vlps", bufs=2, space="PSUM") as lps, \
                    tc.tile_pool(name="vpps", bufs=3, space="PSUM") as pps:
                Wv_s = vstg.tile([128, KT, D], bf16, tag="W")
                load_weight(Wv_d, Wv_s)
                vnT = vstg.tile([128, KT, TH], bf16, tag="xn")
                ln_transpose(xv, TH, vnT, lnw, lps, "vn")
                if not triv_v:
                    bvb = vstg.tile([128, D], bf16, tag="bvb")
                    bvf = vstg.tile([1, D], f32, tag="bvf")
                    nc.sync.dma_start(out=bvf, in_=bvr_d)
                    bvh = vstg.tile([1, D], bf16, tag="bvh")
                    nc.vector.tensor_copy(out=bvh, in_=bvf)
                    nc.gpsimd.partition_broadcast(out_ap=bvb, in_ap=bvh)
                if dedup:
                    vdst = vstg.tile([128, TH // 128, H, HD + 1], bf16,
                                     tag="vh")
                    nc.vector.memset(vdst[:, :, :, HD:HD + 1], 1.0)
                else:
                    vdst = v_aug
                    nc.vector.memset(v_aug[:, :, :, HD:HD + 1], 1.0)

                def v_proj_tt(tt):
                    for n in range(2):
                        ps = pps.tile([128, 512], f32, tag="pj",
                                      name=f"vp{tt}_{n}")
                        for kt in range(KT):
                            nc.tensor.matmul(
                                out=ps,
                                lhsT=vnT[:, kt, tt * 128:(tt + 1) * 128],
                                rhs=Wv_s[:, kt, n * 512:(n + 1) * 512],
                                start=(kt == 0), stop=(kt == KT - 1))
                        if triv_v:
                            nc.vector.tensor_copy(
                                out=vdst[:, tt, n * 8:(n + 1) * 8, 0:HD],
                                in_=ps.rearrange("p (h d) -> p h d", h=8))
                        else:
                            nc.vector.scalar_tensor_tensor(
                                out=vdst[:, tt, n * 8:(n + 1) * 8, 0:HD],
                                in0=ps.rearrange("p (h d) -> p h d", h=8),
                                scalar=1.0, op0=OP.mult, op1=OP.add,
                                in1=bvb[:, n * 512:(n + 1) * 512].rearrange(
                                    "p (h d) -> p h d", h=8))

                if dedup:
                    # halves of the own token range; exchange each half as
                    # soon as it is projected so v arrives early.
                    for half, (vb, vg) in enumerate(
                            ((v_bounce_a, v_gath_a), (v_bounce_b, v_gath_b))):
                        for tt in range(half * 4, half * 4 + 4):
                            v_proj_tt(tt)
                        nc.sync.dma_start(
                            out=vb,
                            in_=vdst[:, half * 4:half * 4 + 4].rearrange(
                                "p t h d -> p (t h d)"))
                        nc.gpsimd.collective_compute(
                            "AllGather", OP.bypass, replica_groups=PAIRS,
                            ins=[vb.opt()], outs=[vg.opt()])
                        for s in range(2):
                            nc.sync.dma_start(
                                out=v_aug[:, s * 8 + half * 4:
                                          s * 8 + half * 4 + 4].rearrange(
                                              "p t h d -> p (t h d)"),
                                in_=vg[s])
                else:
                    for tt in range(TT):
                        v_proj_tt(tt)

            # Wo loads lazily during attention (emitted after pair 0)
            wop = ctx.enter_context(tc.tile_pool(name="wop", bufs=1))
            Wo_s = wop.tile([128, KT, D], bf16)

            # ---------------- attention: pair loop ----------------
            with tc.tile_pool(name="psS", bufs=2, space="PSUM") as psS, \
                    tc.tile_pool(name="psO", bufs=2, space="PSUM") as psO, \
                    tc.tile_pool(name="et", bufs=5) as etp, \
                    tc.tile_pool(name="dv", bufs=1) as dvp:
                for p in range(H // 2):
                    hA, hB = 2 * p, 2 * p + 1
                    # Q-proj block p, just in time (shares psS slots)
                    for n in range(2):
                        psq = psS.tile([128, 512], f32, tag="s",
                                       name=f"qp{p}_{n}")
                        for kt in range(KT):
                            nc.tensor.matmul(
                                out=psq,
                                lhsT=Wq_s[:, kt, p * 128:(p + 1) * 128],
                                rhs=qnT_t[:, kt, n * 512:(n + 1) * 512],
                                start=(kt == 0), stop=(kt == KT - 1))
                        if triv_q:
                            nc.vector.tensor_copy(
                                out=qT_s[:, p, n * 512:(n + 1) * 512],
                                in_=psq)
                        else:
                            nc.scalar.activation(
                                out=qT_s[:, p, n * 512:(n + 1) * 512],
                                in_=psq, func=AF.Identity,
                                bias=bqc_s[:, p:p + 1])
                    nc.vector.tensor_copy(out=qtz[0][0:HD, :],
                                          in_=qT_s[0:HD, p, :])
                    nc.vector.tensor_copy(out=qtz[1][HD:128, :],
                                          in_=qT_s[HD:128, p, :])

                    pOA = psO.tile([65, 2, 512], f32, tag="o",
                                   name=f"poa{p}")
                    pOB = psO.tile([65, 2, 512], f32, tag="o",
                                   name=f"pob{p}")
                    etA = [None] * 4
                    etB = [None] * 4
                    # pair 0 defers A@V by 2 quarters (v arrives mid-stream)
                    # and consumes quarters in order q0,q2,q1,q3 (quarters 0/2
                    # come from the first v exchange, 1/3 from the second).
                    av_order = [0, 2, 1, 3] if (p == 0 and dedup) \
                        else [0, 1, 2, 3]
                    lag = 2 if (p == 0 and dedup) else 1
                    av_done = 0

                    def av_quarter(qi, first, last):
                        # each n-half is its own PSUM bank: every bank's
                        # chain needs its own start/stop
                        for jj in range(4):
                            skq = qi * 4 + jj
                            for n in range(2):
                                nc.tensor.matmul(
                                    out=pOA[:, n, :],
                                    lhsT=v_aug[:, skq, hA, :],
                                    rhs=etA[qi][:, jj,
                                                n * 512:(n + 1) * 512],
                                    start=(first and jj == 0),
                                    stop=(last and jj == 3))
                        for jj in range(4):
                            skq = qi * 4 + jj
                            for n in range(2):
                                nc.tensor.matmul(
                                    out=pOB[:, n, :],
                                    lhsT=v_aug[:, skq, hB, :],
                                    rhs=etB[qi][:, jj,
                                                n * 512:(n + 1) * 512],
                                    start=(first and jj == 0),
                                    stop=(last and jj == 3))

                    for sk in range(16):
                        qtr, j = sk // 4, sk % 4
                        if j == 0:
                            etA[qtr] = etp.tile([128, 4, SQ], bf16, tag="et",
                                                name=f"etA{p}_{qtr}")
                            etB[qtr] = etp.tile([128, 4, SQ], bf16, tag="et",
                                                name=f"etB{p}_{qtr}")
                        psa = psS.tile([128, SQ], f32, tag="s",
                                       name=f"psa{p}_{sk}")
                        psb = psS.tile([128, SQ], f32, tag="s",
                                       name=f"psb{p}_{sk}")
                        for n in range(2):
                            nc.tensor.matmul(
                                out=psa[:, n * 512:(n + 1) * 512],
                                lhsT=kT_s[:, p, sk * 128:(sk + 1) * 128],
                                rhs=qtz[0][:, n * 512:(n + 1) * 512],
                                start=True, stop=True)
                            nc.tensor.matmul(
                                out=psb[:, n * 512:(n + 1) * 512],
                                lhsT=kT_s[:, p, sk * 128:(sk + 1) * 128],
                                rhs=qtz[1][:, n * 512:(n + 1) * 512],
                                start=True, stop=True)
                        nc.scalar.activation(out=etA[qtr][:, j, :], in_=psa,
                                             func=AF.Exp, scale=0.125)
                        nc.scalar.activation(out=etB[qtr][:, j, :], in_=psb,
                                             func=AF.Exp, scale=0.125)
                        while av_done <= qtr - lag:
                            qi = av_order[av_done]
                            av_quarter(qi, av_done == 0, av_done == 3)
                            av_done += 1
                    while av_done < 4:
                        qi = av_order[av_done]
                        av_quarter(qi, av_done == 0, av_done == 3)
                        av_done += 1

                    for h, pO, row0 in ((hA, pOA, 0), (hB, pOB, HD)):
                        rs0 = dvp.tile([1, SQ], f32, tag="r0", name=f"r0{h}")
                        nc.vector.tensor_copy(
                            out=rs0, in_=pO[64:65, :, :].rearrange(
                                "p a b -> p (a b)"))
                        rs = dvp.tile([1, SQ], f32, tag="rs", name=f"rs{h}")
                        nc.vector.reciprocal_approx_fast(out=rs, in_=rs0)
                        rb = dvp.tile([HD, SQ], f32, tag="rb", name=f"rb{h}")
                        nc.gpsimd.partition_broadcast(out_ap=rb, in_ap=rs)
                        nc.vector.tensor_mul(
                            out=attn_oT[row0:row0 + HD, p, :],
                            in0=pO[0:64, :, :].rearrange("p a b -> p (a b)"),
                            in1=rb)

                    if p == 0:
                        load_weight(Wo_d, Wo_s)

            # ---------------- out-proj + gate + final LN ----------------
            with tc.tile_pool(name="o_ps", bufs=8, space="PSUM") as pps, \
                    tc.tile_pool(name="o_w", bufs=4) as work:
                if not triv_o:
                    bof = work.tile([1, D], f32, tag="bof", bufs=1)
                    nc.sync.dma_start(out=bof, in_=bor_d)
                    bo_s = work.tile([1, D], bf16, tag="bos", bufs=1)
                    nc.vector.tensor_copy(out=bo_s, in_=bof)
                if not triv_lno:
                    lnog_b = work.tile([128, D], f32, tag="lng", bufs=1)
                    nc.sync.dma_start(out=lnog_b,
                                      in_=bcast_rows(lnog_d, 128))
                    lnob_b = work.tile([128, D], f32, tag="lnb", bufs=1)
                    nc.sync.dma_start(out=lnob_b,
                                      in_=bcast_rows(lnob_d, 128))
                for tt in range(SQ // 128):
                    # gate folded into the final LN: LN(c*x) =
                    # (x - mean(x)) * c/sqrt(c^2 var(x) + eps) * g + b, c > 0.
                    pss = [pps.tile([128, 512], f32, tag="ops",
                                    name=f"ops{tt}_{n}") for n in range(2)]
                    stats = work.tile([128, 2, 6], f32, tag="st2")
                    for n in range(2):
                        ps = pss[n]
                        for kt in range(KT):
                            nc.tensor.matmul(
                                out=ps,
                                lhsT=attn_oT[:, kt, tt * 128:(tt + 1) * 128],
                                rhs=Wo_s[:, kt, n * 512:(n + 1) * 512],
                                start=(kt == 0),
                                stop=(kt == KT - 1 and triv_o))
                        if not triv_o:
                            nc.tensor.matmul(
                                out=ps, lhsT=ones_row[:, 0:128],
                                rhs=bo_s[:, n * 512:(n + 1) * 512],
                                start=False, stop=True)
                        nc.vector.bn_stats(out=stats[:, n, :], in_=ps)
                    mv = work.tile([128, 2], f32, tag="mv2")
                    nc.vector.bn_aggr(out=mv, in_=stats)
                    gc = gate_s[:, tt:tt + 1]
                    gv = work.tile([128, 1], f32, tag="gv")
                    nc.vector.tensor_mul(out=gv, in0=gc, in1=gc)
                    nc.vector.tensor_mul(out=gv, in0=gv, in1=mv[:, 1:2])
                    rstd = work.tile([128, 1], f32, tag="rs2")
                    nc.scalar.activation(out=rstd, in_=gv,
                                         func=AF.Sqrt, bias=eps_t)
                    nc.vector.reciprocal(out=rstd, in_=rstd)
                    sc = work.tile([128, 1], f32, tag="sc")
                    nc.vector.tensor_mul(out=sc, in0=rstd, in1=gc)
                    mb = work.tile([128, 1], f32, tag="mb")
                    nc.vector.tensor_mul(out=mb, in0=mv[:, 0:1], in1=sc)
                    nc.vector.tensor_scalar_mul(out=mb, in0=mb, scalar1=-1.0)
                    xc = work.tile([128, D], f32, tag="xc2")
                    for n in range(2):
                        nc.scalar.activation(
                            out=xc[:, n * 512:(n + 1) * 512], in_=pss[n],
                            func=AF.Identity, bias=mb, scale=sc)
                    if triv_lno:
                        res = xc
                    else:
                        res = work.tile([128, D], f32, tag="res")
                        nc.vector.tensor_mul(out=res, in0=xc, in1=lnog_b)
                        nc.vector.tensor_add(out=res, in0=res, in1=lnob_b)
                    nc.sync.dma_start(
                        out=out_d[tt * 128:(tt + 1) * 128, :], in_=res)

            if kdbg:
                with tc.tile_pool(name="dbgp", bufs=2) as dbgp:
                    for m in range(KT):
                        ck = dbgp.tile([128, SK], f32, tag="ck")
                        nc.vector.tensor_copy(out=ck, in_=kT_s[:, m, :])
                        nc.sync.dma_start(out=dbg_k[m], in_=ck)
                        cq = dbgp.tile([128, SQ], f32, tag="cq")
                        nc.vector.tensor_copy(out=cq, in_=qT_s[:, m, :])
                        nc.sync.dma_start(out=dbg_q[m], in_=cq)
                        ca = dbgp.tile([128, SQ], f32, tag="ca")
                        nc.vector.tensor_copy(out=ca, in_=attn_oT[:, m, :])
                        nc.sync.dma_start(out=dbg_a[m], in_=ca)
                    for t2 in range(SK // 128):
                        cv = dbgp.tile([128, H * (HD + 1)], f32, tag="cv")
                        nc.vector.tensor_copy(
                            out=cv, in_=v_aug[:, t2].rearrange(
                                "p h d -> p (h d)"))
                        nc.sync.dma_start(out=dbg_v[t2], in_=cv)
                    nc.sync.dma_start(out=dbg_g, in_=gate_s)

    nc.compile()
    return nc


def _maybe_enable_trace():
    """Install the axon NTFF profile hook if tracing was requested."""
    if not os.environ.get("BASS_KERNEL_TRACE"):
        return False
    try:
        import sys
        import types
        import antenv
        if "antenv.axon_hooks" not in sys.modules:
            mod = types.ModuleType("antenv.axon_hooks")
            mod._hook = None
            mod.set_axon_ntff_profile_hook = lambda h: setattr(mod, "_hook", h)
            mod.get_axon_ntff_profile_hook = lambda: mod._hook
            sys.modules["antenv.axon_hooks"] = mod
            antenv.axon_hooks = mod
        from antenv.axon_hooks import get_axon_ntff_profile_hook
        if get_axon_ntff_profile_hook() is None:
            from trn_agent_boot.trn_boot import _ntff_profile_via_ctypes
            from antenv.axon_hooks import set_axon_ntff_profile_hook
            set_axon_ntff_profile_hook(
                _ntff_profile_via_ctypes("/opt/axon/libaxon_pjrt.so"))
        return True
    except Exception:
        return False


def kernel(**inputs):
    from concourse import bass_utils

    f = lambda k: np.ascontiguousarray(np.asarray(inputs[k], dtype=np.float32))
    dedup = True
    TH = SK // 2 if dedup else SK

    import ml_dtypes
    bft = ml_dtypes.bfloat16

    q_g, kv_g = f("ln_q_g"), f("ln_kv_g")
    q_b, kv_b = f("ln_q_b"), f("ln_kv_b")
    Wq_r, Wk_r, Wv_r = f("Wq"), f("Wk"), f("Wv")
    Wg_r = f("Wg").reshape(D, 1)
    # host-side fold: W' = diag(g) @ W, b' = beta @ W + b; ship bf16
    Wq = np.ascontiguousarray((q_g[:, None] * Wq_r).astype(bft))
    Wk = np.ascontiguousarray((kv_g[:, None] * Wk_r).astype(bft))
    Wv = np.ascontiguousarray((kv_g[:, None] * Wv_r).astype(bft))
    Wg = np.ascontiguousarray(
        (q_g * Wg_r[:, 0]).reshape(KT, 128).astype(bft))
    bq = q_b @ Wq_r + f("bq")
    bk = kv_b @ Wk_r + f("bk")
    bv = kv_b @ Wv_r + f("bv")
    bg = (q_b @ Wg_r).reshape(1, 1)
    bqc = np.ascontiguousarray(bq.reshape(KT, 128).T)
    bkc = np.ascontiguousarray(bk.reshape(KT, 128).T)
    bvr = bv.reshape(1, D)
    bor = f("bo").reshape(1, D)
    lnog = f("ln_o_g").reshape(1, D)
    lnob = f("ln_o_b").reshape(1, D)

    triv_k = not bkc.any()
    triv_q = not (bqc.any() or bg.any())
    triv_v = not bvr.any()
    triv_o = not bor.any()
    triv_lno = (not lnob.any()) and bool((lnog == 1.0).all())

    key = ("nc", dedup, triv_k, triv_q, triv_v, triv_o, triv_lno)
    if key not in _CACHE:
        _CACHE[key] = _build(dedup, triv_k, triv_q, triv_v, triv_o, triv_lno)
    nc = _CACHE[key]

    query, kkey, value = f("query"), f("key"), f("value")
    shared = {
        "Wq": Wq, "Wk": Wk, "Wv": Wv,
        "Wo": np.ascontiguousarray(f("Wo").astype(bft)), "Wg": Wg,
        "bqc": bqc, "bkc": bkc, "bvr": np.ascontiguousarray(bvr),
        "bor": np.ascontiguousarray(bor), "bg": np.ascontiguousarray(bg),
        "lnog": np.ascontiguousarray(lnog), "lnob": np.ascontiguousarray(lnob),
    }
    in_maps = []
    for c in range(N_CORES):
        b, hh = c // 2, c % 2
        in_maps.append({
            "xq": np.ascontiguousarray(query[b, hh * SQ:(hh + 1) * SQ, :]),
            "xk": np.ascontiguousarray(
                kkey[b, hh * TH:(hh + 1) * TH, :] if dedup else kkey[b]),
            "xv": np.ascontiguousarray(
                value[b, hh * TH:(hh + 1) * TH, :] if dedup else value[b]),
            **shared,
        })

    trace = _maybe_enable_trace()
    kw = {}
    if trace:
        kw = dict(trace=True, trace_cores=[0])
    res = bass_utils.run_bass_kernel_spmd(
        nc, in_maps, core_ids=list(range(N_CORES)), **kw)
    if trace:
        _CACHE["exec_time_ns"] = res.exec_time_ns
        _CACHE["trace_path"] = (res.instructions_and_trace[1]
                                if res.instructions_and_trace else None)
    if os.environ.get("KDBG"):
        _CACHE["res"] = res.results

    out = np.empty((B, S, D), dtype=np.float32)
    for c in range(N_CORES):
        b, hh = c // 2, c % 2
        out[b, hh * SQ:(hh + 1) * SQ, :] = res.results[c]["out"]
    return out


# revision 36
# speedup vs baseline: 1.0862x; 1.0862x over previous
"""EnhancedMultiHeadAttention on 8 TRN2 NeuronCores — v2.

Sharding: core c = (batch b=c//2, half hh=c%2) owns query rows
hh*1024:(hh+1)*1024 and the SAME kv token half. k/v projections are computed
for the own token half only and exchanged with the pair core via pairwise
AllGather collectives (removes the baseline's duplicated k/v projection work).

Kernel structure per core (bf16 matmuls, f32 softmax/LN):
  - LN gain/beta and projection biases folded on the HOST (W' = diag(g)W,
    b' = beta@W + b); the device only applies bias paths when nonzero.
  - Scores run as TWO CONCURRENT K=64 row-tiled matmuls (head pair on
    partition halves 0:64 / 64:128 of the same kT/qT block) — 2x the
    padded-K=128 baseline throughput.
  - exp on the scalar engine (the attention-phase pacer, ~261us); program
    order is arranged so the exp stream starts as early as possible and
    projections/DMA/exchange overlap it.
  - A@V uses V with a ones column appended (M=65) so the softmax denominator
    accumulates for free in PSUM row 64; normalization via
    reciprocal_approx_fast + gpsimd partition broadcast.
  - Gate is folded into the final LN scale (LN(c*x) trick from baseline).
"""

import os
import numpy as np

D = 1024
H = 16
HD = 64
S = 2048
B = 4
SQ = 1024       # query rows per core
SK = 2048       # kv rows per batch
KT = D // 128   # contraction tiles
N_CORES = 8
EPS = 1e-5
PAIRS = [[0, 1], [2, 3], [4, 5], [6, 7]]

_CACHE = {}


def _build(dedup=True, triv_k=True, triv_q=True, triv_v=True, triv_o=True,
           triv_lno=True):
    from contextlib import ExitStack

    import concourse.bacc as bacc
    import concourse.bass as bass
    import concourse.mybir as mybir
    import concourse.tile as tile
    from concourse.masks import make_identity

    f32 = mybir.dt.float32
    bf16 = mybir.dt.bfloat16
    AF = mybir.ActivationFunctionType
    OP = mybir.AluOpType

    TH = SK // 2 if dedup else SK  # kv tokens projected locally
    TT = SK // 128                 # global kv token tiles (16)
    VROW = H * (HD + 1)            # 1040

    nc = bacc.Bacc("TRN2", target_bir_lowering=False, debug=False,
                   num_devices=N_CORES)

    xq = nc.dram_tensor("xq", [SQ, D], f32, kind="ExternalInput").ap()
    xk = nc.dram_tensor("xk", [TH, D], f32, kind="ExternalInput").ap()
    xv = nc.dram_tensor("xv", [TH, D], f32, kind="ExternalInput").ap()
    # weights arrive pre-cast to bf16 (device would cast anyway): halves the
    # weight DMA traffic and removes the cast ops + staging SBUF entirely.
    Wq_d = nc.dram_tensor("Wq", [D, D], bf16, kind="ExternalInput").ap()
    Wk_d = nc.dram_tensor("Wk", [D, D], bf16, kind="ExternalInput").ap()
    Wv_d = nc.dram_tensor("Wv", [D, D], bf16, kind="ExternalInput").ap()
    Wo_d = nc.dram_tensor("Wo", [D, D], bf16, kind="ExternalInput").ap()
    Wg_d = nc.dram_tensor("Wg", [KT, 128], bf16, kind="ExternalInput").ap()
    bqc_d = nc.dram_tensor("bqc", [128, KT], f32, kind="ExternalInput").ap()
    bkc_d = nc.dram_tensor("bkc", [128, KT], f32, kind="ExternalInput").ap()
    bvr_d = nc.dram_tensor("bvr", [1, D], f32, kind="ExternalInput").ap()
    bor_d = nc.dram_tensor("bor", [1, D], f32, kind="ExternalInput").ap()
    bg_d = nc.dram_tensor("bg", [1, 1], f32, kind="ExternalInput").ap()
    lnog_d = nc.dram_tensor("lnog", [1, D], f32, kind="ExternalInput").ap()
    lnob_d = nc.dram_tensor("lnob", [1, D], f32, kind="ExternalInput").ap()
    out_d = nc.dram_tensor("out", [SQ, D], f32, kind="ExternalOutput").ap()
    kdbg = bool(os.environ.get("KDBG"))
    if kdbg:
        dbg_k = nc.dram_tensor("dbg_k", [KT, 128, SK], f32,
                               kind="ExternalOutput").ap()
        dbg_q = nc.dram_tensor("dbg_q", [KT, 128, SQ], f32,
                               kind="ExternalOutput").ap()
        dbg_v = nc.dram_tensor("dbg_v", [SK // 128, 128, H * (HD + 1)], f32,
                               kind="ExternalOutput").ap()
        dbg_a = nc.dram_tensor("dbg_a", [KT, 128, SQ], f32,
                               kind="ExternalOutput").ap()
        dbg_g = nc.dram_tensor("dbg_g", [128, SQ // 128], f32,
                               kind="ExternalOutput").ap()

    def bcast_rows(ap2d, p):
        return bass.AP(tensor=ap2d.tensor, offset=ap2d.offset,
                       ap=[[0, p]] + list(ap2d.ap[1:]))

    with tile.TileContext(nc) as tc:
        with ExitStack() as ctx:
            const = ctx.enter_context(tc.tile_pool(name="const", bufs=1))
            main = ctx.enter_context(tc.tile_pool(name="main", bufs=1))

            identity = const.tile([128, 128], bf16)
            make_identity(nc, identity)
            eps_t = const.tile([128, 1], f32)
            nc.vector.memset(eps_t, EPS)
            ones_row = const.tile([1, 512], bf16)
            nc.vector.memset(ones_row, 1.0)
            if not triv_k:
                bkc_s = const.tile([128, KT], f32)
                nc.sync.dma_start(out=bkc_s, in_=bkc_d)
            if not triv_q:
                bqc_s = const.tile([128, KT], f32)
                nc.sync.dma_start(out=bqc_s, in_=bqc_d)
                bgf = const.tile([1, 1], f32)
                nc.sync.dma_start(out=bgf, in_=bg_d)
                bg_s = const.tile([1, 1], bf16)
                nc.vector.tensor_copy(out=bg_s, in_=bgf)

            # persistent per-core tiles
            kT_s = main.tile([128, KT, SK], bf16)
            qT_s = main.tile([128, KT, SQ], bf16)
            v_aug = main.tile([128, TT, H, HD + 1], bf16)
            attn_oT = main.tile([128, KT, SQ], bf16)
            gate_s = main.tile([128, SQ // 128], f32)

            if dedup:
                dram = ctx.enter_context(
                    tc.tile_pool(name="dram", bufs=1, space="DRAM"))
                k_bounce = [dram.tile([128, KT // 2 * TH], bf16,
                                      name=f"kb{i}") for i in range(2)]
                k_gath = [dram.tile([2, 128, KT // 2 * TH], bf16,
                                    name=f"kg{i}") for i in range(2)]
                v_bounce_a = dram.tile([128, 4 * VROW], bf16)
                v_gath_a = dram.tile([2, 128, 4 * VROW], bf16)
                v_bounce_b = dram.tile([128, 4 * VROW], bf16)
                v_gath_b = dram.tile([2, 128, 4 * VROW], bf16)

            def load_weight(w_dram, Ws):
                # direct bf16 DMA: Ws[p, k, d] = W[k*128+p, d]
                nc.sync.dma_start(
                    out=Ws, in_=w_dram.rearrange("(k p) d -> p k d", p=128))

            def ln_transpose(x_dram, n_tok, dst, lnw, lps, name):
                """LN (no gain/beta) + PE transpose into dst [128, KT, n_tok]."""
                for t in range(n_tok // 128):
                    xt = lnw.tile([128, D], f32, tag="x", name=f"{name}x{t}")
                    nc.sync.dma_start(
                        out=xt, in_=x_dram[t * 128:(t + 1) * 128, :])
                    xt3 = xt.rearrange("p (s f) -> p s f", s=2)
                    stats = lnw.tile([128, 2, 6], f32, tag="st")
                    nc.vector.bn_stats(out=stats[:, 0, :], in_=xt3[:, 0, :])
                    nc.vector.bn_stats(out=stats[:, 1, :], in_=xt3[:, 1, :])
                    mv = lnw.tile([128, 2], f32, tag="mv")
                    nc.vector.bn_aggr(out=mv, in_=stats)
                    rstd = lnw.tile([128, 1], f32, tag="rs")
                    nc.scalar.activation(out=rstd, in_=mv[:, 1:2],
                                         func=AF.Sqrt, bias=eps_t)
                    nc.vector.reciprocal(out=rstd, in_=rstd)
                    xc = lnw.tile([128, D], bf16, tag="xc", bufs=2)
                    nc.vector.tensor_scalar(
                        out=xc, in0=xt, scalar1=mv[:, 0:1], scalar2=rstd,
                        op0=OP.subtract, op1=OP.mult)
                    pt = lps.tile([128, KT, 128], bf16, tag="pt")
                    for c in range(KT):
                        nc.tensor.transpose(
                            out=pt[:, c, :],
                            in_=xc[:, c * 128:(c + 1) * 128],
                            identity=identity)
                    nc.scalar.copy(out=dst[:, :, t * 128:(t + 1) * 128],
                                   in_=pt)

            # ---------------- K path ----------------
            with tc.tile_pool(name="kstg", bufs=1) as kstg, \
                    tc.tile_pool(name="klnw", bufs=3) as lnw, \
                    tc.tile_pool(name="klps", bufs=2, space="PSUM") as lps, \
                    tc.tile_pool(name="kpps", bufs=3, space="PSUM") as pps:
                Wk_s = kstg.tile([128, KT, D], bf16, tag="W")
                load_weight(Wk_d, Wk_s)
                knT = kstg.tile([128, KT, TH], bf16, tag="xn")
                ln_transpose(xk, TH, knT, lnw, lps, "kn")
                dstk = kstg.tile([128, KT, TH], bf16, tag="kh",
                                 name="kT_half") if dedup else kT_s
                # exchange in two halves (m 0-3, m 4-7) so pair-0 scores can
                # start as soon as the first half lands.
                for half in range(2):
                    for m in range(half * KT // 2, (half + 1) * KT // 2):
                        for n in range(TH // 512):
                            ps = pps.tile([128, 512], f32, tag="pj",
                                          name=f"kp{m}_{n}")
                            for kt in range(KT):
                                nc.tensor.matmul(
                                    out=ps,
                                    lhsT=Wk_s[:, kt, m * 128:(m + 1) * 128],
                                    rhs=knT[:, kt, n * 512:(n + 1) * 512],
                                    start=(kt == 0), stop=(kt == KT - 1))
                            if triv_k:
                                nc.vector.tensor_copy(
                                    out=dstk[:, m, n * 512:(n + 1) * 512],
                                    in_=ps)
                            else:
                                nc.scalar.activation(
                                    out=dstk[:, m, n * 512:(n + 1) * 512],
                                    in_=ps, func=AF.Identity,
                                    bias=bkc_s[:, m:m + 1])
                        if dedup:
                            # exchange DMAs ride the gpsimd (SWDGE) queue so
                            # their collective-semaphore waits never block
                            # the SP queue that feeds input/weight loads
                            mm = m - half * KT // 2
                            nc.gpsimd.dma_start(
                                out=k_bounce[half][:, mm * TH:(mm + 1) * TH],
                                in_=dstk[:, m, :])
                    if dedup:
                        nc.gpsimd.collective_compute(
                            "AllGather", OP.bypass, replica_groups=PAIRS,
                            ins=[k_bounce[half].opt()],
                            outs=[k_gath[half].opt()])
                        kg = k_gath[half].rearrange("s p (m t) -> p m s t",
                                                    m=KT // 2)
                        for mm in range(KT // 2):
                            nc.gpsimd.dma_start(
                                out=kT_s[:, half * KT // 2 + mm, :].rearrange(
                                    "p (s t) -> p s t", s=2),
                                in_=kg[:, mm])

            # ---------------- Q prep (proj happens inside pair loop) -------
            qper = ctx.enter_context(tc.tile_pool(name="qper", bufs=1))
            qnT_t = qper.tile([128, KT, SQ], bf16)
            Wq_s = qper.tile([128, KT, D], bf16)
            Wg_s = qper.tile([128, KT], bf16)
            # zero-padded per-parity q staging: scores run K=128 against the
            # full two-head kT block (other head's rows hit zeros). K=64
            # matmuls get HAM-throttled to half clock — padding is faster.
            qtz = [qper.tile([128, SQ], bf16, name=f"qtz{i}")
                   for i in range(2)]
            nc.vector.memset(qtz[0], 0.0)
            nc.vector.memset(qtz[1], 0.0)
            with tc.tile_pool(name="qlnw", bufs=3) as lnw, \
                    tc.tile_pool(name="qlps", bufs=2, space="PSUM") as lps, \
                    tc.tile_pool(name="gps", bufs=1, space="PSUM") as gps:
                load_weight(Wq_d, Wq_s)
                nc.sync.dma_start(out=Wg_s,
                                  in_=Wg_d.rearrange("k p -> p k"))
                ln_transpose(xq, SQ, qnT_t, lnw, lps, "qn")
                for tt in range(SQ // 128):
                    ps = gps.tile([128, 1], f32, tag="g", name=f"g{tt}")
                    for kt in range(KT):
                        nc.tensor.matmul(
                            out=ps,
                            lhsT=qnT_t[:, kt, tt * 128:(tt + 1) * 128],
                            rhs=Wg_s[:, kt:kt + 1],
                            start=(kt == 0), stop=(kt == KT - 1 and triv_q))
                    if not triv_q:
                        nc.tensor.matmul(out=ps, lhsT=ones_row[:, 0:128],
                                         rhs=bg_s, start=False, stop=True)
                    nc.scalar.activation(out=gate_s[:, tt:tt + 1], in_=ps,
                                         func=AF.Sigmoid)

            # ---------------- V path ----------------
            with tc.tile_pool(name="vstg", bufs=1) as vstg, \
                    tc.tile_pool(name="vlnw", bufs=3) as lnw, \
                    tc.tile_pool(name="vlps", bufs=2, space="PSUM") as lps, \
                    tc.tile_pool(name="vpps", bufs=3, space="PSUM") as pps:
                Wv_s = vstg.tile([128, KT, D], bf16, tag="W")
                load_weight(Wv_d, Wv_s)
                vnT = vstg.tile([128, KT, TH], bf16, tag="xn")
                ln_transpose(xv, TH, vnT, lnw, lps, "vn")
                if not triv_v:
                    bvb = vstg.tile([128, D], bf16, tag="bvb")
                    bvf = vstg.tile([1, D], f32, tag="bvf")
                    nc.sync.dma_start(out=bvf, in_=bvr_d)
                    bvh = vstg.tile([1, D], bf16, tag="bvh")
                    nc.vector.tensor_copy(out=bvh, in_=bvf)
                    nc.gpsimd.partition_broadcast(out_ap=bvb, in_ap=bvh)
                if dedup:
                    vdst = vstg.tile([128, TH // 128, H, HD + 1], bf16,
                                     tag="vh")
                    nc.vector.memset(vdst[:, :, :, HD:HD + 1], 1.0)
                else:
                    vdst = v_aug
                    nc.vector.memset(v_aug[:, :, :, HD:HD + 1], 1.0)

                def v_proj_tt(tt):
                    for n in range(2):
                        ps = pps.tile([128, 512], f32, tag="pj",
                                      name=f"vp{tt}_{n}")
                        for kt in range(KT):
                            nc.tensor.matmul(
                                out=ps,
                                lhsT=vnT[:, kt, tt * 128:(tt + 1) * 128],
                                rhs=Wv_s[:, kt, n * 512:(n + 1) * 512],
                                start=(kt == 0), stop=(kt == KT - 1))
                        if triv_v:
                            nc.vector.tensor_copy(
                                out=vdst[:, tt, n * 8:(n + 1) * 8, 0:HD],
                                in_=ps.rearrange("p (h d) -> p h d", h=8))
                        else:
                            nc.vector.scalar_tensor_tensor(
                                out=vdst[:, tt, n * 8:(n + 1) * 8, 0:HD],
                                in0=ps.rearrange("p (h d) -> p h d", h=8),
                                scalar=1.0, op0=OP.mult, op1=OP.add,
                                in1=bvb[:, n * 512:(n + 1) * 512].rearrange(
                                    "p (h d) -> p h d", h=8))

                if dedup:
                    # halves of the own token range; exchange each half as
                    # soon as it is projected so v arrives early.
                    for half, (vb, vg) in enumerate(
                            ((v_bounce_a, v_gath_a), (v_bounce_b, v_gath_b))):
                        for tt in range(half * 4, half * 4 + 4):
                            v_proj_tt(tt)
                        nc.gpsimd.dma_start(
                            out=vb,
                            in_=vdst[:, half * 4:half * 4 + 4].rearrange(
                                "p t h d -> p (t h d)"))
                        nc.gpsimd.collective_compute(
                            "AllGather", OP.bypass, replica_groups=PAIRS,
                            ins=[vb.opt()], outs=[vg.opt()])
                        for s in range(2):
                            nc.gpsimd.dma_start(
                                out=v_aug[:, s * 8 + half * 4:
                                          s * 8 + half * 4 + 4].rearrange(
                                              "p t h d -> p (t h d)"),
                                in_=vg[s])
                else:
                    for tt in range(TT):
                        v_proj_tt(tt)

            # Wo loads lazily during attention (emitted after pair 0)
            wop = ctx.enter_context(tc.tile_pool(name="wop", bufs=1))
            Wo_s = wop.tile([128, KT, D], bf16)

            # ---------------- attention: pair loop ----------------
            with tc.tile_pool(name="psS", bufs=2, space="PSUM") as psS, \
                    tc.tile_pool(name="psO", bufs=2, space="PSUM") as psO, \
                    tc.tile_pool(name="et", bufs=5) as etp, \
                    tc.tile_pool(name="dv", bufs=1) as dvp:
                for p in range(H // 2):
                    hA, hB = 2 * p, 2 * p + 1
                    # Q-proj block p, just in time (shares psS slots)
                    for n in range(2):
                        psq = psS.tile([128, 512], f32, tag="s",
                                       name=f"qp{p}_{n}")
                        for kt in range(KT):
                            nc.tensor.matmul(
                                out=psq,
                                lhsT=Wq_s[:, kt, p * 128:(p + 1) * 128],
                                rhs=qnT_t[:, kt, n * 512:(n + 1) * 512],
                                start=(kt == 0), stop=(kt == KT - 1))
                        if triv_q:
                            nc.vector.tensor_copy(
                                out=qT_s[:, p, n * 512:(n + 1) * 512],
                                in_=psq)
                        else:
                            nc.scalar.activation(
                                out=qT_s[:, p, n * 512:(n + 1) * 512],
                                in_=psq, func=AF.Identity,
                                bias=bqc_s[:, p:p + 1])
                    nc.vector.tensor_copy(out=qtz[0][0:HD, :],
                                          in_=qT_s[0:HD, p, :])
                    nc.vector.tensor_copy(out=qtz[1][HD:128, :],
                                          in_=qT_s[HD:128, p, :])

                    pOA = psO.tile([65, 2, 512], f32, tag="o",
                                   name=f"poa{p}")
                    pOB = psO.tile([65, 2, 512], f32, tag="o",
                                   name=f"pob{p}")
                    etA = [None] * 4
                    etB = [None] * 4
                    # pair 0 defers A@V by 2 quarters (v arrives mid-stream)
                    # and consumes quarters in order q0,q2,q1,q3 (quarters 0/2
                    # come from the first v exchange, 1/3 from the second).
                    av_order = [0, 2, 1, 3] if (p == 0 and dedup) \
                        else [0, 1, 2, 3]
                    lag = 2 if (p == 0 and dedup) else 1
                    av_done = 0

                    def av_quarter(qi, first, last):
                        # each n-half is its own PSUM bank: every bank's
                        # chain needs its own start/stop
                        for jj in range(4):
                            skq = qi * 4 + jj
                            for n in range(2):
                                nc.tensor.matmul(
                                    out=pOA[:, n, :],
                                    lhsT=v_aug[:, skq, hA, :],
                                    rhs=etA[qi][:, jj,
                                                n * 512:(n + 1) * 512],
                                    start=(first and jj == 0),
                                    stop=(last and jj == 3))
                        for jj in range(4):
                            skq = qi * 4 + jj
                            for n in range(2):
                                nc.tensor.matmul(
                                    out=pOB[:, n, :],
                                    lhsT=v_aug[:, skq, hB, :],
                                    rhs=etB[qi][:, jj,
                                                n * 512:(n + 1) * 512],
                                    start=(first and jj == 0),
                                    stop=(last and jj == 3))

                    for sk in range(16):
                        qtr, j = sk // 4, sk % 4
                        if j == 0:
                            etA[qtr] = etp.tile([128, 4, SQ], bf16, tag="et",
                                                name=f"etA{p}_{qtr}")
                            etB[qtr] = etp.tile([128, 4, SQ], bf16, tag="et",
                                                name=f"etB{p}_{qtr}")
                        psa = psS.tile([128, SQ], f32, tag="s",
                                       name=f"psa{p}_{sk}")
                        psb = psS.tile([128, SQ], f32, tag="s",
                                       name=f"psb{p}_{sk}")
                        for n in range(2):
                            nc.tensor.matmul(
                                out=psa[:, n * 512:(n + 1) * 512],
                                lhsT=kT_s[:, p, sk * 128:(sk + 1) * 128],
                                rhs=qtz[0][:, n * 512:(n + 1) * 512],
                                start=True, stop=True)
                            nc.tensor.matmul(
                                out=psb[:, n * 512:(n + 1) * 512],
                                lhsT=kT_s[:, p, sk * 128:(sk + 1) * 128],
                                rhs=qtz[1][:, n * 512:(n + 1) * 512],
                                start=True, stop=True)
                        nc.scalar.activation(out=etA[qtr][:, j, :], in_=psa,
                                             func=AF.Exp, scale=0.125)
                        nc.scalar.activation(out=etB[qtr][:, j, :], in_=psb,
                                             func=AF.Exp, scale=0.125)
                        while av_done <= qtr - lag:
                            qi = av_order[av_done]
                            av_quarter(qi, av_done == 0, av_done == 3)
                            av_done += 1
                    while av_done < 4:
                        qi = av_order[av_done]
                        av_quarter(qi, av_done == 0, av_done == 3)
                        av_done += 1

                    for h, pO, row0 in ((hA, pOA, 0), (hB, pOB, HD)):
                        rs0 = dvp.tile([1, SQ], f32, tag="r0", name=f"r0{h}")
                        nc.vector.tensor_copy(
                            out=rs0, in_=pO[64:65, :, :].rearrange(
                                "p a b -> p (a b)"))
                        rs = dvp.tile([1, SQ], f32, tag="rs", name=f"rs{h}")
                        nc.vector.reciprocal_approx_fast(out=rs, in_=rs0)
                        rb = dvp.tile([HD, SQ], f32, tag="rb", name=f"rb{h}")
                        nc.gpsimd.partition_broadcast(out_ap=rb, in_ap=rs)
                        nc.vector.tensor_mul(
                            out=attn_oT[row0:row0 + HD, p, :],
                            in0=pO[0:64, :, :].rearrange("p a b -> p (a b)"),
                            in1=rb)

                    if p == 0:
                        load_weight(Wo_d, Wo_s)

            # ---------------- out-proj + gate + final LN ----------------
            with tc.tile_pool(name="o_ps", bufs=8, space="PSUM") as pps, \
                    tc.tile_pool(name="o_w", bufs=4) as work:
                if not triv_o:
                    bof = work.tile([1, D], f32, tag="bof", bufs=1)
                    nc.sync.dma_start(out=bof, in_=bor_d)
                    bo_s = work.tile([1, D], bf16, tag="bos", bufs=1)
                    nc.vector.tensor_copy(out=bo_s, in_=bof)
                if not triv_lno:
                    lnog_b = work.tile([128, D], f32, tag="lng", bufs=1)
                    nc.sync.dma_start(out=lnog_b,
                                      in_=bcast_rows(lnog_d, 128))
                    lnob_b = work.tile([128, D], f32, tag="lnb", bufs=1)
                    nc.sync.dma_start(out=lnob_b,
                                      in_=bcast_rows(lnob_d, 128))
                for tt in range(SQ // 128):
                    # gate folded into the final LN: LN(c*x) =
                    # (x - mean(x)) * c/sqrt(c^2 var(x) + eps) * g + b, c > 0.
                    pss = [pps.tile([128, 512], f32, tag="ops",
                                    name=f"ops{tt}_{n}") for n in range(2)]
                    stats = work.tile([128, 2, 6], f32, tag="st2")
                    for n in range(2):
                        ps = pss[n]
                        for kt in range(KT):
                            nc.tensor.matmul(
                                out=ps,
                                lhsT=attn_oT[:, kt, tt * 128:(tt + 1) * 128],
                                rhs=Wo_s[:, kt, n * 512:(n + 1) * 512],
                                start=(kt == 0),
                                stop=(kt == KT - 1 and triv_o))
                        if not triv_o:
                            nc.tensor.matmul(
                                out=ps, lhsT=ones_row[:, 0:128],
                                rhs=bo_s[:, n * 512:(n + 1) * 512],
                                start=False, stop=True)
                        nc.vector.bn_stats(out=stats[:, n, :], in_=ps)
                    mv = work.tile([128, 2], f32, tag="mv2")
                    nc.vector.bn_aggr(out=mv, in_=stats)
                    gc = gate_s[:, tt:tt + 1]
                    gv = work.tile([128, 1], f32, tag="gv")
                    nc.vector.tensor_mul(out=gv, in0=gc, in1=gc)
                    nc.vector.tensor_mul(out=gv, in0=gv, in1=mv[:, 1:2])
                    rstd = work.tile([128, 1], f32, tag="rs2")
                    nc.scalar.activation(out=rstd, in_=gv,
                                         func=AF.Sqrt, bias=eps_t)
                    nc.vector.reciprocal(out=rstd, in_=rstd)
                    sc = work.tile([128, 1], f32, tag="sc")
                    nc.vector.tensor_mul(out=sc, in0=rstd, in1=gc)
                    mb = work.tile([128, 1], f32, tag="mb")
                    nc.vector.tensor_mul(out=mb, in0=mv[:, 0:1], in1=sc)
                    nc.vector.tensor_scalar_mul(out=mb, in0=mb, scalar1=-1.0)
                    xc = work.tile([128, D], f32, tag="xc2")
                    for n in range(2):
                        nc.scalar.activation(
                            out=xc[:, n * 512:(n + 1) * 512], in_=pss[n],
                            func=AF.Identity, bias=mb, scale=sc)
                    if triv_lno:
                        res = xc
                    else:
                        res = work.tile([128, D], f32, tag="res")
                        nc.vector.tensor_mul(out=res, in0=xc, in1=lnog_b)
                        nc.vector.tensor_add(out=res, in0=res, in1=lnob_b)
                    nc.sync.dma_start(
                        out=out_d[tt * 128:(tt + 1) * 128, :], in_=res)

            if kdbg:
                with tc.tile_pool(name="dbgp", bufs=2) as dbgp:
                    for m in range(KT):
                        ck = dbgp.tile([128, SK], f32, tag="ck")
                        nc.vector.tensor_copy(out=ck, in_=kT_s[:, m, :])
                        nc.sync.dma_start(out=dbg_k[m], in_=ck)
                        cq = dbgp.tile([128, SQ], f32, tag="cq")
                        nc.vector.tensor_copy(out=cq, in_=qT_s[:, m, :])
                        nc.sync.dma_start(out=dbg_q[m], in_=cq)
                        ca = dbgp.tile([128, SQ], f32, tag="ca")
                        nc.vector.tensor_copy(out=ca, in_=attn_oT[:, m, :])
                        nc.sync.dma_start(out=dbg_a[m], in_=ca)
                    for t2 in range(SK // 128):
                        cv = dbgp.tile([128, H * (HD + 1)], f32, tag="cv")
                        nc.vector.tensor_copy(
                            out=cv, in_=v_aug[:, t2].rearrange(
                                "p h d -> p (h d)"))
                        nc.sync.dma_start(out=dbg_v[t2], in_=cv)
                    nc.sync.dma_start(out=dbg_g, in_=gate_s)

    nc.compile()
    return nc


def _maybe_enable_trace():
    """Install the axon NTFF profile hook if tracing was requested."""
    if not os.environ.get("BASS_KERNEL_TRACE"):
        return False
    try:
        import sys
        import types
        import antenv
        if "antenv.axon_hooks" not in sys.modules:
            mod = types.ModuleType("antenv.axon_hooks")
            mod._hook = None
            mod.set_axon_ntff_profile_hook = lambda h: setattr(mod, "_hook", h)
            mod.get_axon_ntff_profile_hook = lambda: mod._hook
            sys.modules["antenv.axon_hooks"] = mod
            antenv.axon_hooks = mod
        from antenv.axon_hooks import get_axon_ntff_profile_hook
        if get_axon_ntff_profile_hook() is None:
            from trn_agent_boot.trn_boot import _ntff_profile_via_ctypes
            from antenv.axon_hooks import set_axon_ntff_profile_hook
            set_axon_ntff_profile_hook(
                _ntff_profile_via_ctypes("/opt/axon/libaxon_pjrt.so"))
        return True
    except Exception:
        return False


def kernel(**inputs):
    from concourse import bass_utils

    f = lambda k: np.ascontiguousarray(np.asarray(inputs[k], dtype=np.float32))
    dedup = True
    TH = SK // 2 if dedup else SK

    import ml_dtypes
    bft = ml_dtypes.bfloat16

    q_g, kv_g = f("ln_q_g"), f("ln_kv_g")
    q_b, kv_b = f("ln_q_b"), f("ln_kv_b")
    Wq_r, Wk_r, Wv_r = f("Wq"), f("Wk"), f("Wv")
    Wg_r = f("Wg").reshape(D, 1)
    # host-side fold: W' = diag(g) @ W, b' = beta @ W + b; ship bf16
    Wq = np.ascontiguousarray((q_g[:, None] * Wq_r).astype(bft))
    Wk = np.ascontiguousarray((kv_g[:, None] * Wk_r).astype(bft))
    Wv = np.ascontiguousarray((kv_g[:, None] * Wv_r).astype(bft))
    Wg = np.ascontiguousarray(
        (q_g * Wg_r[:, 0]).reshape(KT, 128).astype(bft))
    bq = q_b @ Wq_r + f("bq")
    bk = kv_b @ Wk_r + f("bk")
    bv = kv_b @ Wv_r + f("bv")
    bg = (q_b @ Wg_r).reshape(1, 1)
    bqc = np.ascontiguousarray(bq.reshape(KT, 128).T)
    bkc = np.ascontiguousarray(bk.reshape(KT, 128).T)
    bvr = bv.reshape(1, D)
    bor = f("bo").reshape(1, D)
    lnog = f("ln_o_g").reshape(1, D)
    lnob = f("ln_o_b").reshape(1, D)

    triv_k = not bkc.any()
    triv_q = not (bqc.any() or bg.any())
    triv_v = not bvr.any()
    triv_o = not bor.any()
    triv_lno = (not lnob.any()) and bool((lnog == 1.0).all())

    key = ("nc", dedup, triv_k, triv_q, triv_v, triv_o, triv_lno)
    if key not in _CACHE:
        _CACHE[key] = _build(dedup, triv_k, triv_q, triv_v, triv_o, triv_lno)
    nc = _CACHE[key]

    query, kkey, value = f("query"), f("key"), f("value")
    shared = {
        "Wq": Wq, "Wk": Wk, "Wv": Wv,
        "Wo": np.ascontiguousarray(f("Wo").astype(bft)), "Wg": Wg,
        "bqc": bqc, "bkc": bkc, "bvr": np.ascontiguousarray(bvr),
        "bor": np.ascontiguousarray(bor), "bg": np.ascontiguousarray(bg),
        "lnog": np.ascontiguousarray(lnog), "lnob": np.ascontiguousarray(lnob),
    }
    in_maps = []
    for c in range(N_CORES):
        b, hh = c // 2, c % 2
        in_maps.append({
            "xq": np.ascontiguousarray(query[b, hh * SQ:(hh + 1) * SQ, :]),
            "xk": np.ascontiguousarray(
                kkey[b, hh * TH:(hh + 1) * TH, :] if dedup else kkey[b]),
            "xv": np.ascontiguousarray(
                value[b, hh * TH:(hh + 1) * TH, :] if dedup else value[b]),
            **shared,
        })

    trace = _maybe_enable_trace()
    kw = {}
    if trace:
        kw = dict(trace=True, trace_cores=[0])
    res = bass_utils.run_bass_kernel_spmd(
        nc, in_maps, core_ids=list(range(N_CORES)), **kw)
    if trace:
        _CACHE["exec_time_ns"] = res.exec_time_ns
        _CACHE["trace_path"] = (res.instructions_and_trace[1]
                                if res.instructions_and_trace else None)
    if os.environ.get("KDBG"):
        _CACHE["res"] = res.results

    out = np.empty((B, S, D), dtype=np.float32)
    for c in range(N_CORES):
        b, hh = c // 2, c % 2
        out[b, hh * SQ:(hh + 1) * SQ, :] = res.results[c]["out"]
    return out


# revision 45
# speedup vs baseline: 1.1780x; 1.0846x over previous
"""EnhancedMultiHeadAttention on 8 TRN2 NeuronCores — v2.

Sharding: core c = (batch b=c//2, half hh=c%2) owns query rows
hh*1024:(hh+1)*1024 and the SAME kv token half. k/v projections are computed
for the own token half only and exchanged with the pair core via pairwise
AllGather collectives (removes the baseline's duplicated k/v projection work).

Kernel structure per core (bf16 matmuls, f32 softmax/LN):
  - LN gain/beta and projection biases folded on the HOST (W' = diag(g)W,
    b' = beta@W + b); the device only applies bias paths when nonzero.
  - Scores run as TWO CONCURRENT K=64 row-tiled matmuls (head pair on
    partition halves 0:64 / 64:128 of the same kT/qT block) — 2x the
    padded-K=128 baseline throughput.
  - exp on the scalar engine (the attention-phase pacer, ~261us); program
    order is arranged so the exp stream starts as early as possible and
    projections/DMA/exchange overlap it.
  - A@V uses V with a ones column appended (M=65) so the softmax denominator
    accumulates for free in PSUM row 64; normalization via
    reciprocal_approx_fast + gpsimd partition broadcast.
  - Gate is folded into the final LN scale (LN(c*x) trick from baseline).
"""

import os
import numpy as np

D = 1024
H = 16
HD = 64
S = 2048
B = 4
SQ = 1024       # query rows per core
SK = 2048       # kv rows per batch
KT = D // 128   # contraction tiles
N_CORES = 8
EPS = 1e-5
PAIRS = [[0, 1], [2, 3], [4, 5], [6, 7]]

_CACHE = {}


def _build(dedup=True, triv_k=True, triv_q=True, triv_v=True, triv_o=True,
           triv_lno=True):
    from contextlib import ExitStack

    import concourse.bacc as bacc
    import concourse.bass as bass
    import concourse.mybir as mybir
    import concourse.tile as tile
    from concourse.masks import make_identity

    f32 = mybir.dt.float32
    bf16 = mybir.dt.bfloat16
    AF = mybir.ActivationFunctionType
    OP = mybir.AluOpType

    TH = SK // 2 if dedup else SK  # kv tokens projected locally
    TT = SK // 128                 # global kv token tiles (16)
    VROW = H * (HD + 1)            # 1040

    nc = bacc.Bacc("TRN2", target_bir_lowering=False, debug=False,
                   num_devices=N_CORES)

    xq = nc.dram_tensor("xq", [SQ, D], f32, kind="ExternalInput").ap()
    xk = nc.dram_tensor("xk", [TH, D], f32, kind="ExternalInput").ap()
    xv = nc.dram_tensor("xv", [TH, D], f32, kind="ExternalInput").ap()
    # weights arrive pre-cast to bf16 (device would cast anyway): halves the
    # weight DMA traffic and removes the cast ops + staging SBUF entirely.
    Wq_d = nc.dram_tensor("Wq", [D, D], bf16, kind="ExternalInput").ap()
    Wk_d = nc.dram_tensor("Wk", [D, D], bf16, kind="ExternalInput").ap()
    Wv_d = nc.dram_tensor("Wv", [D, D], bf16, kind="ExternalInput").ap()
    Wo_d = nc.dram_tensor("Wo", [D, D], bf16, kind="ExternalInput").ap()
    Wg_d = nc.dram_tensor("Wg", [KT, 128], bf16, kind="ExternalInput").ap()
    bqc_d = nc.dram_tensor("bqc", [128, KT], f32, kind="ExternalInput").ap()
    bkc_d = nc.dram_tensor("bkc", [128, KT], f32, kind="ExternalInput").ap()
    bvr_d = nc.dram_tensor("bvr", [1, D], f32, kind="ExternalInput").ap()
    bor_d = nc.dram_tensor("bor", [1, D], f32, kind="ExternalInput").ap()
    bg_d = nc.dram_tensor("bg", [1, 1], f32, kind="ExternalInput").ap()
    lnog_d = nc.dram_tensor("lnog", [1, D], f32, kind="ExternalInput").ap()
    lnob_d = nc.dram_tensor("lnob", [1, D], f32, kind="ExternalInput").ap()
    out_d = nc.dram_tensor("out", [SQ, D], f32, kind="ExternalOutput").ap()
    kdbg = bool(os.environ.get("KDBG"))
    if kdbg:
        dbg_k = nc.dram_tensor("dbg_k", [KT, 128, SK], f32,
                               kind="ExternalOutput").ap()
        dbg_q = nc.dram_tensor("dbg_q", [KT, 128, SQ], f32,
                               kind="ExternalOutput").ap()
        dbg_v = nc.dram_tensor("dbg_v", [SK // 128, 128, H * (HD + 1)], f32,
                               kind="ExternalOutput").ap()
        dbg_a = nc.dram_tensor("dbg_a", [KT, 128, SQ], f32,
                               kind="ExternalOutput").ap()
        dbg_g = nc.dram_tensor("dbg_g", [128, SQ // 128], f32,
                               kind="ExternalOutput").ap()

    def bcast_rows(ap2d, p):
        return bass.AP(tensor=ap2d.tensor, offset=ap2d.offset,
                       ap=[[0, p]] + list(ap2d.ap[1:]))

    with tile.TileContext(nc) as tc:
        with ExitStack() as ctx:
            const = ctx.enter_context(tc.tile_pool(name="const", bufs=1))
            main = ctx.enter_context(tc.tile_pool(name="main", bufs=1))

            identity = const.tile([128, 128], bf16)
            make_identity(nc, identity)
            eps_t = const.tile([128, 1], f32)
            nc.vector.memset(eps_t, EPS)
            ones_row = const.tile([1, 512], bf16)
            nc.vector.memset(ones_row, 1.0)
            if not triv_k:
                bkc_s = const.tile([128, KT], f32)
                nc.sync.dma_start(out=bkc_s, in_=bkc_d)
            if not triv_q:
                bqc_s = const.tile([128, KT], f32)
                nc.sync.dma_start(out=bqc_s, in_=bqc_d)
                bgf = const.tile([1, 1], f32)
                nc.sync.dma_start(out=bgf, in_=bg_d)
                bg_s = const.tile([1, 1], bf16)
                nc.vector.tensor_copy(out=bg_s, in_=bgf)

            # persistent per-core tiles
            kT_s = main.tile([128, KT, SK], bf16)
            qT_s = main.tile([128, KT, SQ], bf16)
            v_aug = main.tile([128, TT, H, HD + 1], bf16)
            attn_oT = main.tile([128, KT, SQ], bf16)
            gate_s = main.tile([128, SQ // 128], f32)

            if dedup:
                dram = ctx.enter_context(
                    tc.tile_pool(name="dram", bufs=1, space="DRAM"))
                k_bounce = [dram.tile([128, KT // 2 * TH], bf16,
                                      name=f"kb{i}") for i in range(2)]
                k_gath = [dram.tile([2, 128, KT // 2 * TH], bf16,
                                    name=f"kg{i}") for i in range(2)]
                v_bounce_a = dram.tile([128, 4 * VROW], bf16)
                v_gath_a = dram.tile([2, 128, 4 * VROW], bf16)
                v_bounce_b = dram.tile([128, 4 * VROW], bf16)
                v_gath_b = dram.tile([2, 128, 4 * VROW], bf16)

            def load_weight(w_dram, Ws):
                # direct bf16 DMA: Ws[p, k, d] = W[k*128+p, d]
                nc.sync.dma_start(
                    out=Ws, in_=w_dram.rearrange("(k p) d -> p k d", p=128))

            def emit_k_readback(half):
                kg = k_gath[half].rearrange("s p (m t) -> p m s t",
                                            m=KT // 2)
                for mm in range(KT // 2):
                    nc.gpsimd.dma_start(
                        out=kT_s[:, half * KT // 2 + mm, :].rearrange(
                            "p (s t) -> p s t", s=2),
                        in_=kg[:, mm])

            def ln_transpose(x_dram, n_tok, dst, lnw, lps, name):
                """LN (no gain/beta) + PE transpose into dst [128, KT, n_tok]."""
                for t in range(n_tok // 128):
                    xt = lnw.tile([128, D], f32, tag="x", name=f"{name}x{t}")
                    nc.sync.dma_start(
                        out=xt, in_=x_dram[t * 128:(t + 1) * 128, :])
                    xt3 = xt.rearrange("p (s f) -> p s f", s=2)
                    stats = lnw.tile([128, 2, 6], f32, tag="st")
                    nc.vector.bn_stats(out=stats[:, 0, :], in_=xt3[:, 0, :])
                    nc.vector.bn_stats(out=stats[:, 1, :], in_=xt3[:, 1, :])
                    mv = lnw.tile([128, 2], f32, tag="mv")
                    nc.vector.bn_aggr(out=mv, in_=stats)
                    rstd = lnw.tile([128, 1], f32, tag="rs")
                    nc.scalar.activation(out=rstd, in_=mv[:, 1:2],
                                         func=AF.Sqrt, bias=eps_t)
                    nc.vector.reciprocal(out=rstd, in_=rstd)
                    xc = lnw.tile([128, D], bf16, tag="xc", bufs=2)
                    nc.vector.tensor_scalar(
                        out=xc, in0=xt, scalar1=mv[:, 0:1], scalar2=rstd,
                        op0=OP.subtract, op1=OP.mult)
                    pt = lps.tile([128, KT, 128], bf16, tag="pt")
                    for c in range(KT):
                        nc.tensor.transpose(
                            out=pt[:, c, :],
                            in_=xc[:, c * 128:(c + 1) * 128],
                            identity=identity)
                    nc.scalar.copy(out=dst[:, :, t * 128:(t + 1) * 128],
                                   in_=pt)

            # ---------------- K path ----------------
            with tc.tile_pool(name="kstg", bufs=1) as kstg, \
                    tc.tile_pool(name="klnw", bufs=3) as lnw, \
                    tc.tile_pool(name="klps", bufs=2, space="PSUM") as lps, \
                    tc.tile_pool(name="kpps", bufs=3, space="PSUM") as pps:
                Wk_s = kstg.tile([128, KT, D], bf16, tag="W")
                load_weight(Wk_d, Wk_s)
                knT = kstg.tile([128, KT, TH], bf16, tag="xn")
                ln_transpose(xk, TH, knT, lnw, lps, "kn")
                dstk = kstg.tile([128, KT, TH], bf16, tag="kh",
                                 name="kT_half") if dedup else kT_s
                # exchange in two halves (m 0-3, m 4-7) so pair-0 scores can
                # start as soon as the first half lands. All bounce DMAs and
                # collective triggers are emitted BEFORE any readback: the
                # readbacks wait on collective semaphores and would otherwise
                # head-of-line-block later triggers on the gpsimd queue.
                for half in range(2):
                    for m in range(half * KT // 2, (half + 1) * KT // 2):
                        for n in range(TH // 512):
                            ps = pps.tile([128, 512], f32, tag="pj",
                                          name=f"kp{m}_{n}")
                            for kt in range(KT):
                                nc.tensor.matmul(
                                    out=ps,
                                    lhsT=Wk_s[:, kt, m * 128:(m + 1) * 128],
                                    rhs=knT[:, kt, n * 512:(n + 1) * 512],
                                    start=(kt == 0), stop=(kt == KT - 1))
                            if triv_k:
                                nc.vector.tensor_copy(
                                    out=dstk[:, m, n * 512:(n + 1) * 512],
                                    in_=ps)
                            else:
                                nc.scalar.activation(
                                    out=dstk[:, m, n * 512:(n + 1) * 512],
                                    in_=ps, func=AF.Identity,
                                    bias=bkc_s[:, m:m + 1])
                        if dedup:
                            mm = m - half * KT // 2
                            nc.gpsimd.dma_start(
                                out=k_bounce[half][:, mm * TH:(mm + 1) * TH],
                                in_=dstk[:, m, :])
                    if dedup:
                        nc.gpsimd.collective_compute(
                            "AllGather", OP.bypass, replica_groups=PAIRS,
                            ins=[k_bounce[half].opt()],
                            outs=[k_gath[half].opt()])
                if dedup:
                    emit_k_readback(0)  # half 1 readback deferred to V phase

            # ---------------- Q prep (proj happens inside pair loop) -------
            qper = ctx.enter_context(tc.tile_pool(name="qper", bufs=1))
            qnT_t = qper.tile([128, KT, SQ], bf16)
            Wq_s = qper.tile([128, KT, D], bf16)
            Wg_s = qper.tile([128, KT], bf16)
            # zero-padded per-parity q staging: scores run K=128 against the
            # full two-head kT block (other head's rows hit zeros). K=64
            # matmuls get HAM-throttled to half clock — padding is faster.
            qtz = [qper.tile([128, SQ], bf16, name=f"qtz{i}")
                   for i in range(2)]
            nc.vector.memset(qtz[0], 0.0)
            nc.vector.memset(qtz[1], 0.0)
            def q_proj_m(p, psq_pool, tag):
                for n in range(2):
                    psq = psq_pool.tile([128, 512], f32, tag=tag,
                                        name=f"qp{p}_{n}")
                    for kt in range(KT):
                        nc.tensor.matmul(
                            out=psq,
                            lhsT=Wq_s[:, kt, p * 128:(p + 1) * 128],
                            rhs=qnT_t[:, kt, n * 512:(n + 1) * 512],
                            start=(kt == 0), stop=(kt == KT - 1))
                    if triv_q:
                        nc.vector.tensor_copy(
                            out=qT_s[:, p, n * 512:(n + 1) * 512], in_=psq)
                    else:
                        nc.scalar.activation(
                            out=qT_s[:, p, n * 512:(n + 1) * 512],
                            in_=psq, func=AF.Identity,
                            bias=bqc_s[:, p:p + 1])

            with tc.tile_pool(name="qlnw", bufs=3) as lnw, \
                    tc.tile_pool(name="qlps", bufs=2, space="PSUM") as lps, \
                    tc.tile_pool(name="gps", bufs=1, space="PSUM") as gps:
                load_weight(Wq_d, Wq_s)
                nc.sync.dma_start(out=Wg_s,
                                  in_=Wg_d.rearrange("k p -> p k"))
                ln_transpose(xq, SQ, qnT_t, lnw, lps, "qn")
                q_proj_m(0, gps, "qp")
                q_proj_m(1, gps, "qp")
                for tt in range(SQ // 128):
                    ps = gps.tile([128, 1], f32, tag="g", name=f"g{tt}")
                    for kt in range(KT):
                        nc.tensor.matmul(
                            out=ps,
                            lhsT=qnT_t[:, kt, tt * 128:(tt + 1) * 128],
                            rhs=Wg_s[:, kt:kt + 1],
                            start=(kt == 0), stop=(kt == KT - 1 and triv_q))
                    if not triv_q:
                        nc.tensor.matmul(out=ps, lhsT=ones_row[:, 0:128],
                                         rhs=bg_s, start=False, stop=True)
                    nc.scalar.activation(out=gate_s[:, tt:tt + 1], in_=ps,
                                         func=AF.Sigmoid)

            # ---------------- V path ----------------
            with tc.tile_pool(name="vstg", bufs=1) as vstg, \
                    tc.tile_pool(name="vlnw", bufs=3) as lnw, \
                    tc.tile_pool(name="vlps", bufs=2, space="PSUM") as lps, \
                    tc.tile_pool(name="vpps", bufs=3, space="PSUM") as pps:
                Wv_s = vstg.tile([128, KT, D], bf16, tag="W")
                load_weight(Wv_d, Wv_s)
                vnT = vstg.tile([128, KT, TH], bf16, tag="xn")
                ln_transpose(xv, TH, vnT, lnw, lps, "vn")
                if not triv_v:
                    bvb = vstg.tile([128, D], bf16, tag="bvb")
                    bvf = vstg.tile([1, D], f32, tag="bvf")
                    nc.sync.dma_start(out=bvf, in_=bvr_d)
                    bvh = vstg.tile([1, D], bf16, tag="bvh")
                    nc.vector.tensor_copy(out=bvh, in_=bvf)
                    nc.gpsimd.partition_broadcast(out_ap=bvb, in_ap=bvh)
                if dedup:
                    vdst = vstg.tile([128, TH // 128, H, HD + 1], bf16,
                                     tag="vh")
                    nc.vector.memset(vdst[:, :, :, HD:HD + 1], 1.0)
                else:
                    vdst = v_aug
                    nc.vector.memset(v_aug[:, :, :, HD:HD + 1], 1.0)

                def v_proj_tt(tt):
                    for n in range(2):
                        ps = pps.tile([128, 512], f32, tag="pj",
                                      name=f"vp{tt}_{n}")
                        for kt in range(KT):
                            nc.tensor.matmul(
                                out=ps,
                                lhsT=vnT[:, kt, tt * 128:(tt + 1) * 128],
                                rhs=Wv_s[:, kt, n * 512:(n + 1) * 512],
                                start=(kt == 0), stop=(kt == KT - 1))
                        if triv_v:
                            nc.vector.tensor_copy(
                                out=vdst[:, tt, n * 8:(n + 1) * 8, 0:HD],
                                in_=ps.rearrange("p (h d) -> p h d", h=8))
                        else:
                            nc.vector.scalar_tensor_tensor(
                                out=vdst[:, tt, n * 8:(n + 1) * 8, 0:HD],
                                in0=ps.rearrange("p (h d) -> p h d", h=8),
                                scalar=1.0, op0=OP.mult, op1=OP.add,
                                in1=bvb[:, n * 512:(n + 1) * 512].rearrange(
                                    "p (h d) -> p h d", h=8))

                if dedup:
                    # halves of the own token range; exchange each half as
                    # soon as it is projected so v arrives early. Triggers
                    # first, readbacks after (see K path comment). The
                    # deferred k half-1 readback rides between the v
                    # triggers so it can't delay them.
                    for half, (vb, vg) in enumerate(
                            ((v_bounce_a, v_gath_a), (v_bounce_b, v_gath_b))):
                        for tt in range(half * 4, half * 4 + 4):
                            v_proj_tt(tt)
                        nc.gpsimd.dma_start(
                            out=vb,
                            in_=vdst[:, half * 4:half * 4 + 4].rearrange(
                                "p t h d -> p (t h d)"))
                        nc.gpsimd.collective_compute(
                            "AllGather", OP.bypass, replica_groups=PAIRS,
                            ins=[vb.opt()], outs=[vg.opt()])
                        if half == 0:
                            emit_k_readback(1)
                    for half, vg in enumerate((v_gath_a, v_gath_b)):
                        for s in range(2):
                            nc.gpsimd.dma_start(
                                out=v_aug[:, s * 8 + half * 4:
                                          s * 8 + half * 4 + 4].rearrange(
                                              "p t h d -> p (t h d)"),
                                in_=vg[s])
                else:
                    for tt in range(TT):
                        v_proj_tt(tt)

            # Wo loads lazily during attention (emitted after pair 0)
            wop = ctx.enter_context(tc.tile_pool(name="wop", bufs=1))
            Wo_s = wop.tile([128, KT, D], bf16)

            # ---------------- attention: pair loop ----------------
            with tc.tile_pool(name="psS", bufs=2, space="PSUM") as psS, \
                    tc.tile_pool(name="psO", bufs=2, space="PSUM") as psO, \
                    tc.tile_pool(name="et", bufs=5) as etp, \
                    tc.tile_pool(name="dv", bufs=1) as dvp:
                for p in range(H // 2):
                    hA, hB = 2 * p, 2 * p + 1
                    nc.vector.tensor_copy(out=qtz[0][0:HD, :],
                                          in_=qT_s[0:HD, p, :])
                    nc.vector.tensor_copy(out=qtz[1][HD:128, :],
                                          in_=qT_s[HD:128, p, :])

                    pOA = psO.tile([65, 2, 512], f32, tag="o",
                                   name=f"poa{p}")
                    pOB = psO.tile([65, 2, 512], f32, tag="o",
                                   name=f"pob{p}")
                    etA = [None] * 4
                    etB = [None] * 4
                    # early pairs defer A@V (v arrives mid-stream) and
                    # consume quarters in order q0,q2,q1,q3: quarters 0/2 are
                    # the token ranges from the first v exchange, 1/3 from
                    # the second.
                    av_order = [0, 2, 1, 3] if (p == 0 and dedup) \
                        else [0, 1, 2, 3]
                    lag = 2 if (p == 0 and dedup) else 1
                    av_done = 0

                    def av_quarter(qi, first, last):
                        # each n-half is its own PSUM bank: every bank's
                        # chain needs its own start/stop
                        for jj in range(4):
                            skq = qi * 4 + jj
                            for n in range(2):
                                nc.tensor.matmul(
                                    out=pOA[:, n, :],
                                    lhsT=v_aug[:, skq, hA, :],
                                    rhs=etA[qi][:, jj,
                                                n * 512:(n + 1) * 512],
                                    start=(first and jj == 0),
                                    stop=(last and jj == 3))
                        for jj in range(4):
                            skq = qi * 4 + jj
                            for n in range(2):
                                nc.tensor.matmul(
                                    out=pOB[:, n, :],
                                    lhsT=v_aug[:, skq, hB, :],
                                    rhs=etB[qi][:, jj,
                                                n * 512:(n + 1) * 512],
                                    start=(first and jj == 0),
                                    stop=(last and jj == 3))

                    for sk in range(16):
                        qtr, j = sk // 4, sk % 4
                        if j == 0:
                            etA[qtr] = etp.tile([128, 4, SQ], bf16, tag="et",
                                                name=f"etA{p}_{qtr}")
                            etB[qtr] = etp.tile([128, 4, SQ], bf16, tag="et",
                                                name=f"etB{p}_{qtr}")
                        psa = psS.tile([128, SQ], f32, tag="s",
                                       name=f"psa{p}_{sk}")
                        psb = psS.tile([128, SQ], f32, tag="s",
                                       name=f"psb{p}_{sk}")
                        for n in range(2):
                            nc.tensor.matmul(
                                out=psa[:, n * 512:(n + 1) * 512],
                                lhsT=kT_s[:, p, sk * 128:(sk + 1) * 128],
                                rhs=qtz[0][:, n * 512:(n + 1) * 512],
                                start=True, stop=True)
                            nc.tensor.matmul(
                                out=psb[:, n * 512:(n + 1) * 512],
                                lhsT=kT_s[:, p, sk * 128:(sk + 1) * 128],
                                rhs=qtz[1][:, n * 512:(n + 1) * 512],
                                start=True, stop=True)
                        nc.scalar.activation(out=etA[qtr][:, j, :], in_=psa,
                                             func=AF.Exp, scale=0.125)
                        nc.scalar.activation(out=etB[qtr][:, j, :], in_=psb,
                                             func=AF.Exp, scale=0.125)
                        while av_done <= qtr - lag:
                            qi = av_order[av_done]
                            av_quarter(qi, av_done == 0, av_done == 3)
                            av_done += 1
                    while av_done < 4:
                        qi = av_order[av_done]
                        av_quarter(qi, av_done == 0, av_done == 3)
                        av_done += 1

                    for h, pO, row0 in ((hA, pOA, 0), (hB, pOB, HD)):
                        rs0 = dvp.tile([1, SQ], f32, tag="r0", name=f"r0{h}")
                        nc.vector.tensor_copy(
                            out=rs0, in_=pO[64:65, :, :].rearrange(
                                "p a b -> p (a b)"))
                        rs = dvp.tile([1, SQ], f32, tag="rs", name=f"rs{h}")
                        nc.vector.reciprocal_approx_fast(out=rs, in_=rs0)
                        rb = dvp.tile([HD, SQ], f32, tag="rb", name=f"rb{h}")
                        nc.gpsimd.partition_broadcast(out_ap=rb, in_ap=rs)
                        nc.vector.tensor_mul(
                            out=attn_oT[row0:row0 + HD, p, :],
                            in0=pO[0:64, :, :].rearrange("p a b -> p (a b)"),
                            in1=rb)

                    if p == 0:
                        load_weight(Wo_d, Wo_s)
                    # stage the (p+2)-th Q projection into the PE slack of
                    # this EXP-paced pair
                    if p < H // 2 - 2:
                        q_proj_m(p + 2, psS, "s")

            # ---------------- out-proj + gate + final LN ----------------
            with tc.tile_pool(name="o_ps", bufs=8, space="PSUM") as pps, \
                    tc.tile_pool(name="o_w", bufs=4) as work:
                if not triv_o:
                    bof = work.tile([1, D], f32, tag="bof", bufs=1)
                    nc.sync.dma_start(out=bof, in_=bor_d)
                    bo_s = work.tile([1, D], bf16, tag="bos", bufs=1)
                    nc.vector.tensor_copy(out=bo_s, in_=bof)
                if not triv_lno:
                    lnog_b = work.tile([128, D], f32, tag="lng", bufs=1)
                    nc.sync.dma_start(out=lnog_b,
                                      in_=bcast_rows(lnog_d, 128))
                    lnob_b = work.tile([128, D], f32, tag="lnb", bufs=1)
                    nc.sync.dma_start(out=lnob_b,
                                      in_=bcast_rows(lnob_d, 128))
                for tt in range(SQ // 128):
                    # gate folded into the final LN: LN(c*x) =
                    # (x - mean(x)) * c/sqrt(c^2 var(x) + eps) * g + b, c > 0.
                    pss = [pps.tile([128, 512], f32, tag="ops",
                                    name=f"ops{tt}_{n}") for n in range(2)]
                    stats = work.tile([128, 2, 6], f32, tag="st2")
                    for n in range(2):
                        ps = pss[n]
                        for kt in range(KT):
                            nc.tensor.matmul(
                                out=ps,
                                lhsT=attn_oT[:, kt, tt * 128:(tt + 1) * 128],
                                rhs=Wo_s[:, kt, n * 512:(n + 1) * 512],
                                start=(kt == 0),
                                stop=(kt == KT - 1 and triv_o))
                        if not triv_o:
                            nc.tensor.matmul(
                                out=ps, lhsT=ones_row[:, 0:128],
                                rhs=bo_s[:, n * 512:(n + 1) * 512],
                                start=False, stop=True)
                        nc.vector.bn_stats(out=stats[:, n, :], in_=ps)
                    mv = work.tile([128, 2], f32, tag="mv2")
                    nc.vector.bn_aggr(out=mv, in_=stats)
                    gc = gate_s[:, tt:tt + 1]
                    gv = work.tile([128, 1], f32, tag="gv")
                    nc.vector.tensor_mul(out=gv, in0=gc, in1=gc)
                    nc.vector.tensor_mul(out=gv, in0=gv, in1=mv[:, 1:2])
                    rstd = work.tile([128, 1], f32, tag="rs2")
                    nc.scalar.activation(out=rstd, in_=gv,
                                         func=AF.Sqrt, bias=eps_t)
                    nc.vector.reciprocal(out=rstd, in_=rstd)
                    sc = work.tile([128, 1], f32, tag="sc")
                    nc.vector.tensor_mul(out=sc, in0=rstd, in1=gc)
                    mb = work.tile([128, 1], f32, tag="mb")
                    nc.vector.tensor_mul(out=mb, in0=mv[:, 0:1], in1=sc)
                    nc.vector.tensor_scalar_mul(out=mb, in0=mb, scalar1=-1.0)
                    xc = work.tile([128, D], f32, tag="xc2")
                    for n in range(2):
                        nc.scalar.activation(
                            out=xc[:, n * 512:(n + 1) * 512], in_=pss[n],
                            func=AF.Identity, bias=mb, scale=sc)
                    if triv_lno:
                        res = xc
                    else:
                        res = work.tile([128, D], f32, tag="res")
                        nc.vector.tensor_mul(out=res, in0=xc, in1=lnog_b)
                        nc.vector.tensor_add(out=res, in0=res, in1=lnob_b)
                    nc.sync.dma_start(
                        out=out_d[tt * 128:(tt + 1) * 128, :], in_=res)

            if kdbg:
                with tc.tile_pool(name="dbgp", bufs=2) as dbgp:
                    for m in range(KT):
                        ck = dbgp.tile([128, SK], f32, tag="ck")
                        nc.vector.tensor_copy(out=ck, in_=kT_s[:, m, :])
                        nc.sync.dma_start(out=dbg_k[m], in_=ck)
                        cq = dbgp.tile([128, SQ], f32, tag="cq")
                        nc.vector.tensor_copy(out=cq, in_=qT_s[:, m, :])
                        nc.sync.dma_start(out=dbg_q[m], in_=cq)
                        ca = dbgp.tile([128, SQ], f32, tag="ca")
                        nc.vector.tensor_copy(out=ca, in_=attn_oT[:, m, :])
                        nc.sync.dma_start(out=dbg_a[m], in_=ca)
                    for t2 in range(SK // 128):
                        cv = dbgp.tile([128, H * (HD + 1)], f32, tag="cv")
                        nc.vector.tensor_copy(
                            out=cv, in_=v_aug[:, t2].rearrange(
                                "p h d -> p (h d)"))
                        nc.sync.dma_start(out=dbg_v[t2], in_=cv)
                    nc.sync.dma_start(out=dbg_g, in_=gate_s)

    nc.compile()
    return nc


def _maybe_enable_trace():
    """Install the axon NTFF profile hook if tracing was requested."""
    if not os.environ.get("BASS_KERNEL_TRACE"):
        return False
    try:
        import sys
        import types
        import antenv
        if "antenv.axon_hooks" not in sys.modules:
            mod = types.ModuleType("antenv.axon_hooks")
            mod._hook = None
            mod.set_axon_ntff_profile_hook = lambda h: setattr(mod, "_hook", h)
            mod.get_axon_ntff_profile_hook = lambda: mod._hook
            sys.modules["antenv.axon_hooks"] = mod
            antenv.axon_hooks = mod
        from antenv.axon_hooks import get_axon_ntff_profile_hook
        if get_axon_ntff_profile_hook() is None:
            from trn_agent_boot.trn_boot import _ntff_profile_via_ctypes
            from antenv.axon_hooks import set_axon_ntff_profile_hook
            set_axon_ntff_profile_hook(
                _ntff_profile_via_ctypes("/opt/axon/libaxon_pjrt.so"))
        return True
    except Exception:
        return False


def kernel(**inputs):
    from concourse import bass_utils

    f = lambda k: np.ascontiguousarray(np.asarray(inputs[k], dtype=np.float32))
    dedup = True
    TH = SK // 2 if dedup else SK

    import ml_dtypes
    bft = ml_dtypes.bfloat16

    q_g, kv_g = f("ln_q_g"), f("ln_kv_g")
    q_b, kv_b = f("ln_q_b"), f("ln_kv_b")
    Wq_r, Wk_r, Wv_r = f("Wq"), f("Wk"), f("Wv")
    Wg_r = f("Wg").reshape(D, 1)
    # host-side fold: W' = diag(g) @ W, b' = beta @ W + b; ship bf16
    Wq = np.ascontiguousarray((q_g[:, None] * Wq_r).astype(bft))
    Wk = np.ascontiguousarray((kv_g[:, None] * Wk_r).astype(bft))
    Wv = np.ascontiguousarray((kv_g[:, None] * Wv_r).astype(bft))
    Wg = np.ascontiguousarray(
        (q_g * Wg_r[:, 0]).reshape(KT, 128).astype(bft))
    bq = q_b @ Wq_r + f("bq")
    bk = kv_b @ Wk_r + f("bk")
    bv = kv_b @ Wv_r + f("bv")
    bg = (q_b @ Wg_r).reshape(1, 1)
    bqc = np.ascontiguousarray(bq.reshape(KT, 128).T)
    bkc = np.ascontiguousarray(bk.reshape(KT, 128).T)
    bvr = bv.reshape(1, D)
    bor = f("bo").reshape(1, D)
    lnog = f("ln_o_g").reshape(1, D)
    lnob = f("ln_o_b").reshape(1, D)

    triv_k = not bkc.any()
    triv_q = not (bqc.any() or bg.any())
    triv_v = not bvr.any()
    triv_o = not bor.any()
    triv_lno = (not lnob.any()) and bool((lnog == 1.0).all())

    key = ("nc", dedup, triv_k, triv_q, triv_v, triv_o, triv_lno)
    if key not in _CACHE:
        _CACHE[key] = _build(dedup, triv_k, triv_q, triv_v, triv_o, triv_lno)
    nc = _CACHE[key]

    query, kkey, value = f("query"), f("key"), f("value")
    shared = {
        "Wq": Wq, "Wk": Wk, "Wv": Wv,
        "Wo": np.ascontiguousarray(f("Wo").astype(bft)), "Wg": Wg,
        "bqc": bqc, "bkc": bkc, "bvr": np.ascontiguousarray(bvr),
        "bor": np.ascontiguousarray(bor), "bg": np.ascontiguousarray(bg),
        "lnog": np.ascontiguousarray(lnog), "lnob": np.ascontiguousarray(lnob),
    }
    in_maps = []
    for c in range(N_CORES):
        b, hh = c // 2, c % 2
        in_maps.append({
            "xq": np.ascontiguousarray(query[b, hh * SQ:(hh + 1) * SQ, :]),
            "xk": np.ascontiguousarray(
                kkey[b, hh * TH:(hh + 1) * TH, :] if dedup else kkey[b]),
            "xv": np.ascontiguousarray(
                value[b, hh * TH:(hh + 1) * TH, :] if dedup else value[b]),
            **shared,
        })

    trace = _maybe_enable_trace()
    kw = {}
    if trace:
        kw = dict(trace=True, trace_cores=[0])
    res = bass_utils.run_bass_kernel_spmd(
        nc, in_maps, core_ids=list(range(N_CORES)), **kw)
    if trace:
        _CACHE["exec_time_ns"] = res.exec_time_ns
        _CACHE["trace_path"] = (res.instructions_and_trace[1]
                                if res.instructions_and_trace else None)
    if os.environ.get("KDBG"):
        _CACHE["res"] = res.results

    out = np.empty((B, S, D), dtype=np.float32)
    for c in range(N_CORES):
        b, hh = c // 2, c % 2
        out[b, hh * SQ:(hh + 1) * SQ, :] = res.results[c]["out"]
    return out
